# revision 3
# baseline (speedup 1.0000x reference)
"""Cubic B-spline evaluation on 8 Trainium2 NeuronCores. v4.

y = C_q(vc) + gamma_q * relu(vc)^3, vc = x - 2q - 1.  5-value payload
(C0..C3, gamma).  MM1: bf16 x-rows vs thresholds 2i; indicators 2-taus-wide
(ACT Sign / DVE {0,2}, engine chosen per pair); MM2: two DoubleRow fp8
matmuls (e4m3 hi/lo + e5m2 lo2/lo3 on bitcast sigma bytes).

Layout: pt = s*32768 + tau*512 + c, tau = G*8 + pr*2 + h;
pointwise p = s*32 + pr*8 + G, f = h*512 + c; psum2 row = val*16 + s*4 + pr.
"""

import sys

sys.path.insert(0, "/opt/trn_rl_repo")

import numpy as np

N_TOTAL = 1_048_576
N_CORES = 8
N = N_TOTAL // N_CORES
P = 128
F = N // P
NS = 4
NT = 64
NG = 8
NPR = 4
TW = 512
SLOTN = N // NS
MAGIC = 8388608.0
EPS = 2.0 ** -14
NV = 5  # payload values
MROWS = NV * 16  # psum2 rows

# engine for each of the 32 indicator pair-ops: pair index = G*4 + (h*2 + prpair)
# True = ACT (sign convention), False = DVE ({0,2})
ENG_ACT = [None] * 32
for _G in range(NG):
    for _h in range(2):
        for _pp in range(2):
            i = _G * 4 + _h * 2 + _pp
            # ~20 ACT / 12 DVE
            ENG_ACT[i] = not (_h == 1 and (_pp == 1 or _G % 2 == 0))
_PROG_CACHE: dict = {}


def _tables(coefs: np.ndarray):
    import ml_dtypes

    E4 = ml_dtypes.float8_e4m3fn
    E5 = ml_dtypes.float8_e5m2

    c = np.zeros(67, np.float64)
    c[3:] = np.asarray(coefs, np.float64)
    jj = np.arange(64)
    a0 = (c[jj] + 4 * c[jj + 1] + c[jj + 2]) / 6
    a1 = (c[jj + 2] - c[jj]) / 2
    a2 = (c[jj] - 2 * c[jj + 1] + c[jj + 2]) / 2
    a3 = (c[jj + 3] - c[jj] + 3 * c[jj + 1] - 3 * c[jj + 2]) / 6
    A = np.stack([a0, a1, a2, a3], 1)

    B = A.copy()
    r1 = jj % 2 == 1
    B[r1, 0] = A[r1, 0] - A[r1, 1] + A[r1, 2] - A[r1, 3]
    B[r1, 1] = A[r1, 1] - 2 * A[r1, 2] + 3 * A[r1, 3]
    B[r1, 2] = A[r1, 2] - 3 * A[r1, 3]
    B[r1, 3] = A[r1, 3]

    def recenter(T):
        o = T.copy()
        o[:, 0] = T[:, 0] + T[:, 1] + T[:, 2] + T[:, 3]
        o[:, 1] = T[:, 1] + 2 * T[:, 2] + 3 * T[:, 3]
        o[:, 2] = T[:, 2] + 3 * T[:, 3]
        o[:, 3] = T[:, 3]
        return o

    C = recenter(B[0::2])
    D = recenter(B[1::2]) - C
    tables = np.column_stack([C, D[:, 3]])  # [32, 5]

    qs = np.arange(32)
    PhiS = np.ones((32, 32))
    Phi0 = np.zeros((32, 32))
    Phi0[:, 0] = 2.0
    for i in range(1, 32):
        PhiS[:, i] = np.where(qs >= i, 1.0, -1.0)
        Phi0[:, i] = np.where(qs >= i, 2.0, 0.0)

    def qq(x, t):
        return np.asarray(x, np.float32).astype(t).astype(np.float64)

    def split4(W, e5scale):
        p1 = qq(W, E4)
        r = W - p1
        p2 = qq(r, E4)
        r = r - p2
        p3 = qq(r * e5scale, E5)
        r = r - p3 / e5scale
        p4 = qq(r * e5scale, E5)
        return p1, p2, p3, p4

    out = {}
    for conv, Phi, e5s in (("sign", PhiS, 2.0), ("02", Phi0, 1.0)):
        W = np.linalg.solve(Phi, tables)
        out[conv] = split4(W, e5s)
    return out


def _host_arrays(coefs):
    import ml_dtypes

    E4 = ml_dtypes.float8_e4m3fn
    E5 = ml_dtypes.float8_e5m2
    bf = ml_dtypes.bfloat16
    sp = _tables(coefs)

    w1 = np.zeros((5, 128), np.float64)
    for s in range(NS):
        for i in range(32):
            col = s * 32 + i
            w1[s, col] = 1.0
            w1[4, col] = -2.0 * i
    w1_bf = w1.astype(np.float32).astype(bf)

    # MM2 lhsT per (conv, pr): [128, 2, MROWS]
    def mk(parts, pr):
        p1, p2, p3, p4 = parts
        a = np.zeros((128, 2, MROWS), np.float64)
        b = np.zeros((128, 2, MROWS), np.float64)
        for s in range(NS):
            for i in range(32):
                k = s * 32 + i
                for val in range(NV):
                    m = val * 16 + s * 4 + pr
                    a[k, 0, m] = p1[i, val]
                    a[k, 1, m] = p2[i, val]
                    b[k, 0, m] = p3[i, val]
                    b[k, 1, m] = p4[i, val]
        return a.astype(np.float32).astype(E4), b.astype(np.float32).astype(E5)

    w2a = np.zeros((2, 4, 128, 2, MROWS), E4)  # [conv, pr, ...]
    w2b = np.zeros((2, 4, 128, 2, MROWS), E5)
    for ci, conv in enumerate(("sign", "02")):
        for pr in range(NPR):
            a, b = mk(sp[conv], pr)
            w2a[ci, pr] = a
            w2b[ci, pr] = b
    return w1_bf, w2a.reshape(8, 128, 2, MROWS), w2b.reshape(8, 128, 2, MROWS)


def _unpermute_y(yp):
    v = yp.reshape(NS, NPR, NG, 2, TW).transpose(0, 2, 1, 3, 4)
    return np.ascontiguousarray(v.reshape(N))


def _build_program():
    import concourse.bacc as bacc
    import concourse.mybir as mybir
    from concourse.tile import TileContext

    f32 = mybir.dt.float32
    f16 = mybir.dt.float16
    bf16 = mybir.dt.bfloat16
    fp8e4 = mybir.dt.float8e4
    fp8e5 = mybir.dt.float8e5
    Alu = mybir.AluOpType
    DR = mybir.MatmulPerfMode.DoubleRow
    AF = mybir.ActivationFunctionType

    nc = bacc.Bacc("TRN2", debug=False)

    x_dram = nc.dram_tensor("x", [N], f32, kind="ExternalInput")
    w1_dram = nc.dram_tensor("w1", [5, 128], bf16, kind="ExternalInput")
    w2a_dram = nc.dram_tensor("w2a", [8, 128, 2, MROWS], fp8e4, kind="ExternalInput")
    w2b_dram = nc.dram_tensor("w2b", [8, 128, 2, MROWS], fp8e5, kind="ExternalInput")
    ones_dram = nc.dram_tensor("ones1", [1, SLOTN], bf16, kind="ExternalInput")
    y_dram = nc.dram_tensor("out", [P, F], f16, kind="ExternalOutput")

    with TileContext(nc) as tc:
        with (
            tc.tile_pool(name="const", bufs=1) as cpool,
            tc.tile_pool(name="pw", bufs=1) as pw,
            tc.tile_pool(name="tmp", bufs=4) as tmp,
            tc.tile_pool(name="sig", bufs=4) as sigp,
            tc.tile_pool(name="stage", bufs=1) as stg,
            tc.tile_pool(name="ps1", bufs=3, space="PSUM") as pp1,
            tc.tile_pool(name="ps2", bufs=1, space="PSUM") as pp2,
        ):
            # ---- earliest: xrows cast (pool) + small consts ----
            xrows = cpool.tile([5, SLOTN], bf16, tag="xrows")
            nc.gpsimd.dma_start(
                out=xrows[0:4, :],
                in_=x_dram.ap().rearrange("(sp t) -> sp t", sp=4),
            )
            w1_sb = cpool.tile([5, 128], bf16, tag="w1")
            nc.sync.dma_start(out=w1_sb[:], in_=w1_dram.ap())
            nc.sync.dma_start(out=xrows[4:5, :], in_=ones_dram.ap())
            eps_sb = cpool.tile([128, 1], f32, tag="eps")
            nc.gpsimd.memset(eps_sb[:], EPS)

            # PE warmup
            psw = pp1.tile([P, 2, TW], f32, tag="s1", name="warm")
            for _ in range(4):
                nc.tensor.matmul(
                    out=psw[:, 0, 0:128], lhsT=w1_sb[:],
                    rhs=w1_sb[:, 0:128], start=True, stop=True,
                )

            # big weights
            w2a_sb = cpool.tile([128, 8, 2, MROWS], fp8e4, tag="w2a")
            nc.sync.dma_start(
                out=w2a_sb[:],
                in_=w2a_dram.ap().rearrange("v k two m -> k v two m"),
            )
            w2b_sb = cpool.tile([128, 8, 2, MROWS], fp8e5, tag="w2b")
            nc.sync.dma_start(
                out=w2b_sb[:],
                in_=w2b_dram.ap().rearrange("v k two m -> k v two m"),
            )

            # ---- pointwise loads + prep (overlaps loop) ----
            x_pw = pw.tile([P, F], f32, tag="x")
            xview = x_dram.ap().rearrange(
                "(sp g pr hh c) -> sp pr g (hh c)", sp=4, g=8, pr=4, hh=2
            )
            for s in range(NS):
                nc.sync.dma_start(out=x_pw[s * 32:(s + 1) * 32, :], in_=xview[s])
            xb_pw = pw.tile([P, F], bf16, tag="xb")
            for s in range(NS):
                nc.gpsimd.dma_start(
                    out=xb_pw[s * 32:(s + 1) * 32, :], in_=xview[s]
                )
            xe_pw = pw.tile([P, F], f32, tag="xe")
            nc.vector.tensor_scalar(
                xe_pw[:], xb_pw[:], 63.75, EPS, Alu.min, Alu.add
            )
            Qb = tmp.tile([P, F], f32, tag="ta", name="Qb")
            nc.scalar.activation(Qb[:], xe_pw[:], AF.Copy,
                                 bias=MAGIC - 0.5, scale=0.5)
            q5 = pw.tile([P, F], f32, tag="q5")
            nc.scalar.activation(q5[:], Qb[:], AF.Copy, bias=-MAGIC + 0.5)
            vc_pw = pw.tile([P, F], f16, tag="vc")
            nc.vector.scalar_tensor_tensor(
                vc_pw[:], q5[:], -2.0, x_pw[:], Alu.mult, Alu.add
            )
            # relu-cube ingredients (ready before tail)
            w_pw = pw.tile([P, F], f16, tag="w")
            nc.vector.tensor_scalar(w_pw[:], vc_pw[:], 0.0, 1.0, Alu.max, Alu.mult)
            w2_pw = pw.tile([P, F], f16, tag="w2")
            nc.vector.tensor_tensor(out=w2_pw[:], in0=w_pw[:], in1=w_pw[:], op=Alu.mult)
            v2_pw = pw.tile([P, F], f16, tag="v2")
            nc.vector.tensor_tensor(out=v2_pw[:], in0=vc_pw[:], in1=vc_pw[:], op=Alu.mult)

            staging = stg.tile([MROWS, NG, 2 * TW], f16, tag="stg")

            # ---- pipelined pair loop: 32 pairs of 2 taus ----
            # pair idx pi = G*4 + h*2 + pp covers taus (G, pr=2pp, h), (G, pr=2pp+1, h)
            pairs = [
                (G, h, pp)
                for G in range(NG)
                for h in range(2)
                for pp in range(2)
            ]
            NPAIR = len(pairs)
            ps1_p = [None] * NPAIR
            sig_p = [None] * NPAIR
            ps2_g = [None] * NG

            def taus_of(pi):
                G, h, pp = pairs[pi]
                return [(G, 2 * pp + d, h) for d in range(2)]

            def s0(pi):  # 2 MM1s into one double tile
                ps1 = pp1.tile([P, 2, TW], f32, tag="s1", name=f"ps1_{pi}")
                ps1_p[pi] = ps1
                for d, (G, pr, h) in enumerate(taus_of(pi)):
                    tau = G * 8 + pr * 2 + h
                    nc.tensor.matmul(
                        out=ps1[:, d], lhsT=w1_sb[:],
                        rhs=xrows[:, tau * TW:(tau + 1) * TW],
                        start=True, stop=True,
                    )

            def s1(pi):  # one 1024-wide indicator
                idx = pairs[pi][0] * 4 + pairs[pi][1] * 2 + pairs[pi][2]
                sig = sigp.tile([P, 2, TW], fp8e4, tag="sg", name=f"sig{pi}")
                sig_p[pi] = sig
                src = ps1_p[pi][:].rearrange("p d c -> p (d c)")
                dst = sig[:].rearrange("p d c -> p (d c)")
                if ENG_ACT[idx]:
                    nc.scalar.activation(dst, src, AF.Sign, bias=eps_sb[:])
                else:
                    nc.vector.tensor_scalar(
                        dst, src, -EPS, 2.0, Alu.is_ge, Alu.mult
                    )

            def s2(pi):  # 4 MM2s + evac at G end
                G, h, pp = pairs[pi]
                idx = G * 4 + h * 2 + pp
                ci = 0 if ENG_ACT[idx] else 1
                if ps2_g[G] is None:
                    ps2_g[G] = pp2.tile([MROWS, 2, TW], f32, tag="s2", name=f"ps2_{G}")
                ps2 = ps2_g[G]
                for d, (G_, pr, h_) in enumerate(taus_of(pi)):
                    wi = ci * 4 + pr
                    sg = sig_p[pi][:, d]
                    rhs2a = sg.unsqueeze(1).broadcast_to([P, 2, TW])
                    nc.tensor.matmul(
                        out=ps2[:, h], lhsT=w2a_sb[:, wi], rhs=rhs2a,
                        start=(pr == 0), stop=False, perf_mode=DR,
                    )
                    rhs2b = sg.bitcast(fp8e5).unsqueeze(1).broadcast_to([P, 2, TW])
                    nc.tensor.matmul(
                        out=ps2[:, h], lhsT=w2b_sb[:, wi], rhs=rhs2b,
                        start=False, stop=(pr == 3), perf_mode=DR,
                    )
                if pp == 1:
                    dst = staging[:, G, h * TW:(h + 1) * TW]
                    src = ps2[:, h]
                    if (G + h) % 2 == 0:
                        nc.scalar.copy(out=dst, in_=src)
                    else:
                        nc.vector.tensor_copy(out=dst, in_=src)

            SKEW = 3
            for t in range(NPAIR + SKEW):
                if 0 <= t - 1 < NPAIR:
                    s1(t - 1)
                if t < NPAIR:
                    s0(t)
                if 0 <= t - SKEW < NPAIR:
                    s2(t - SKEW)

            # ---- reloads + horner ----
            g_pw = pw.tile([P, NV, F], f16, tag="gpw")
            gk = [g_pw[:, v, :] for v in range(NV)]
            for i, val in enumerate((2, 0, 3, 1, 4)):
                eng = nc.gpsimd if i % 2 == 1 else nc.sync
                eng.dma_start(
                    out=g_pw[:, val, :],
                    in_=staging[val * 16:(val + 1) * 16],
                )
            t1 = tmp.tile([P, F], f16, tag="ta", name="t1")
            nc.vector.tensor_tensor(out=t1[:], in0=v2_pw[:], in1=gk[2], op=Alu.mult)
            e0 = tmp.tile([P, F], f16, tag="tb", name="e0")
            nc.vector.tensor_tensor(out=e0[:], in0=gk[0], in1=t1[:], op=Alu.add)
            t2 = tmp.tile([P, F], f16, tag="tc", name="t2")
            nc.vector.tensor_tensor(out=t2[:], in0=v2_pw[:], in1=gk[3], op=Alu.mult)
            e1 = tmp.tile([P, F], f16, tag="td", name="e1")
            nc.vector.tensor_tensor(out=e1[:], in0=gk[1], in1=t2[:], op=Alu.add)
            t4 = tmp.tile([P, F], f16, tag="ta", name="t4")
            nc.vector.tensor_tensor(out=t4[:], in0=gk[4], in1=w_pw[:], op=Alu.mult)
            t3 = tmp.tile([P, F], f16, tag="tc", name="t3")
            nc.vector.tensor_tensor(out=t3[:], in0=vc_pw[:], in1=e1[:], op=Alu.mult)
            y0 = tmp.tile([P, F], f16, tag="td", name="y0")
            nc.vector.tensor_tensor(out=y0[:], in0=e0[:], in1=t3[:], op=Alu.add)
            u = tmp.tile([P, F], f16, tag="tb", name="u")
            nc.vector.tensor_tensor(out=u[:], in0=t4[:], in1=w2_pw[:], op=Alu.mult)
            y16 = pw.tile([P, F], f16, tag="y")
            nc.vector.tensor_tensor(out=y16[:], in0=y0[:], in1=u[:], op=Alu.add)
            nc.sync.dma_start(out=y_dram.ap(), in_=y16[:])

    nc.compile()
    return nc


def get_program():
    if "prog" not in _PROG_CACHE:
        _PROG_CACHE["prog"] = _build_program()
    return _PROG_CACHE["prog"]


def make_in_maps(x: np.ndarray, coefs: np.ndarray):
    import ml_dtypes

    bf = ml_dtypes.bfloat16
    w1, w2a, w2b = _host_arrays(coefs)
    ones1 = np.ones((1, SLOTN), bf)
    shards = np.asarray(x, np.float32).reshape(N_CORES, N)
    return [
        {"x": shards[i].copy(), "w1": w1, "w2a": w2a, "w2b": w2b, "ones1": ones1}
        for i in range(N_CORES)
    ]


def kernel(x, coefs, knot_vector=None, _trace: bool = False):
    from concourse.bass_utils import run_bass_kernel_spmd

    nc = get_program()
    in_maps = make_in_maps(x, coefs)
    res = run_bass_kernel_spmd(nc, in_maps, list(range(N_CORES)), trace=_trace)
    out = np.concatenate(
        [_unpermute_y(r["out"].astype(np.float32)) for r in res.results]
    )
    if _trace:
        return out, res
    return out


# revision 4
# speedup vs baseline: 1.0077x; 1.0077x over previous
"""Cubic B-spline evaluation on 8 Trainium2 NeuronCores. v4.

y = C_q(vc) + gamma_q * relu(vc)^3, vc = x - 2q - 1.  5-value payload
(C0..C3, gamma).  MM1: bf16 x-rows vs thresholds 2i; indicators 2-taus-wide
(ACT Sign / DVE {0,2}, engine chosen per pair); MM2: two DoubleRow fp8
matmuls (e4m3 hi/lo + e5m2 lo2/lo3 on bitcast sigma bytes).

Layout: pt = s*32768 + tau*512 + c, tau = G*8 + pr*2 + h;
pointwise p = s*32 + pr*8 + G, f = h*512 + c; psum2 row = val*16 + s*4 + pr.
"""

import sys

sys.path.insert(0, "/opt/trn_rl_repo")

import numpy as np

N_TOTAL = 1_048_576
N_CORES = 8
N = N_TOTAL // N_CORES
P = 128
F = N // P
NS = 4
NT = 64
NG = 8
NPR = 4
TW = 512
SLOTN = N // NS
MAGIC = 8388608.0
EPS = 2.0 ** -14
NV = 5  # payload values
MROWS = NV * 16  # psum2 rows

# engine for each of the 32 indicator pair-ops: pair index = G*4 + (h*2 + prpair)
# True = ACT (sign convention), False = DVE ({0,2})
ENG_ACT = [None] * 32
for _G in range(NG):
    for _h in range(2):
        for _pp in range(2):
            i = _G * 4 + _h * 2 + _pp
            # ~20 ACT / 12 DVE
            ENG_ACT[i] = not (_h == 1 and (_pp == 1 or _G % 2 == 0))
_PROG_CACHE: dict = {}


def _tables(coefs: np.ndarray):
    import ml_dtypes

    E4 = ml_dtypes.float8_e4m3fn
    E5 = ml_dtypes.float8_e5m2

    c = np.zeros(67, np.float64)
    c[3:] = np.asarray(coefs, np.float64)
    jj = np.arange(64)
    a0 = (c[jj] + 4 * c[jj + 1] + c[jj + 2]) / 6
    a1 = (c[jj + 2] - c[jj]) / 2
    a2 = (c[jj] - 2 * c[jj + 1] + c[jj + 2]) / 2
    a3 = (c[jj + 3] - c[jj] + 3 * c[jj + 1] - 3 * c[jj + 2]) / 6
    A = np.stack([a0, a1, a2, a3], 1)

    B = A.copy()
    r1 = jj % 2 == 1
    B[r1, 0] = A[r1, 0] - A[r1, 1] + A[r1, 2] - A[r1, 3]
    B[r1, 1] = A[r1, 1] - 2 * A[r1, 2] + 3 * A[r1, 3]
    B[r1, 2] = A[r1, 2] - 3 * A[r1, 3]
    B[r1, 3] = A[r1, 3]

    def recenter(T):
        o = T.copy()
        o[:, 0] = T[:, 0] + T[:, 1] + T[:, 2] + T[:, 3]
        o[:, 1] = T[:, 1] + 2 * T[:, 2] + 3 * T[:, 3]
        o[:, 2] = T[:, 2] + 3 * T[:, 3]
        o[:, 3] = T[:, 3]
        return o

    C = recenter(B[0::2])
    D = recenter(B[1::2]) - C
    tables = np.column_stack([C, D[:, 3]])  # [32, 5]

    qs = np.arange(32)
    PhiS = np.ones((32, 32))
    Phi0 = np.zeros((32, 32))
    Phi0[:, 0] = 2.0
    for i in range(1, 32):
        PhiS[:, i] = np.where(qs >= i, 1.0, -1.0)
        Phi0[:, i] = np.where(qs >= i, 2.0, 0.0)

    def qq(x, t):
        return np.asarray(x, np.float32).astype(t).astype(np.float64)

    def split4(W, e5scale):
        p1 = qq(W, E4)
        r = W - p1
        p2 = qq(r, E4)
        r = r - p2
        p3 = qq(r * e5scale, E5)
        r = r - p3 / e5scale
        p4 = qq(r * e5scale, E5)
        return p1, p2, p3, p4

    out = {}
    for conv, Phi, e5s in (("sign", PhiS, 2.0), ("02", Phi0, 1.0)):
        W = np.linalg.solve(Phi, tables)
        out[conv] = split4(W, e5s)
    return out


def _host_arrays(coefs):
    import ml_dtypes

    E4 = ml_dtypes.float8_e4m3fn
    E5 = ml_dtypes.float8_e5m2
    bf = ml_dtypes.bfloat16
    sp = _tables(coefs)

    w1 = np.zeros((5, 128), np.float64)
    for s in range(NS):
        for i in range(32):
            col = s * 32 + i
            w1[s, col] = 1.0
            w1[4, col] = -2.0 * i
    w1_bf = w1.astype(np.float32).astype(bf)

    # MM2 lhsT per (conv, pr): [128, 2, MROWS]
    def mk(parts, pr):
        p1, p2, p3, p4 = parts
        a = np.zeros((128, 2, MROWS), np.float64)
        b = np.zeros((128, 2, MROWS), np.float64)
        for s in range(NS):
            for i in range(32):
                k = s * 32 + i
                for val in range(NV):
                    m = val * 16 + s * 4 + pr
                    a[k, 0, m] = p1[i, val]
                    a[k, 1, m] = p2[i, val]
                    b[k, 0, m] = p3[i, val]
                    b[k, 1, m] = p4[i, val]
        return a.astype(np.float32).astype(E4), b.astype(np.float32).astype(E5)

    w2a = np.zeros((2, 4, 128, 2, MROWS), E4)  # [conv, pr, ...]
    w2b = np.zeros((2, 4, 128, 2, MROWS), E5)
    for ci, conv in enumerate(("sign", "02")):
        for pr in range(NPR):
            a, b = mk(sp[conv], pr)
            w2a[ci, pr] = a
            w2b[ci, pr] = b
    return w1_bf, w2a.reshape(8, 128, 2, MROWS), w2b.reshape(8, 128, 2, MROWS)


def _unpermute_y(yp):
    v = yp.reshape(NS, NPR, NG, 2, TW).transpose(0, 2, 1, 3, 4)
    return np.ascontiguousarray(v.reshape(N))


def _build_program():
    import concourse.bacc as bacc
    import concourse.mybir as mybir
    from concourse.tile import TileContext

    f32 = mybir.dt.float32
    f16 = mybir.dt.float16
    bf16 = mybir.dt.bfloat16
    fp8e4 = mybir.dt.float8e4
    fp8e5 = mybir.dt.float8e5
    Alu = mybir.AluOpType
    DR = mybir.MatmulPerfMode.DoubleRow
    AF = mybir.ActivationFunctionType

    nc = bacc.Bacc("TRN2", debug=False)

    x_dram = nc.dram_tensor("x", [N], f32, kind="ExternalInput")
    w1_dram = nc.dram_tensor("w1", [5, 128], bf16, kind="ExternalInput")
    w2a_dram = nc.dram_tensor("w2a", [8, 128, 2, MROWS], fp8e4, kind="ExternalInput")
    w2b_dram = nc.dram_tensor("w2b", [8, 128, 2, MROWS], fp8e5, kind="ExternalInput")
    ones_dram = nc.dram_tensor("ones1", [1, SLOTN], bf16, kind="ExternalInput")
    y_dram = nc.dram_tensor("out", [P, F], f16, kind="ExternalOutput")

    with TileContext(nc) as tc:
        with (
            tc.tile_pool(name="const", bufs=1) as cpool,
            tc.tile_pool(name="pw", bufs=1) as pw,
            tc.tile_pool(name="tmp", bufs=4) as tmp,
            tc.tile_pool(name="sig", bufs=4) as sigp,
            tc.tile_pool(name="stage", bufs=1) as stg,
            tc.tile_pool(name="ps1", bufs=3, space="PSUM") as pp1,
            tc.tile_pool(name="ps2", bufs=1, space="PSUM") as pp2,
        ):
            # ---- earliest: xrows cast (pool) + small consts ----
            xrows = cpool.tile([5, SLOTN], bf16, tag="xrows")
            nc.gpsimd.dma_start(
                out=xrows[0:4, :],
                in_=x_dram.ap().rearrange("(sp t) -> sp t", sp=4),
            )
            w1_sb = cpool.tile([5, 128], bf16, tag="w1")
            nc.sync.dma_start(out=w1_sb[:], in_=w1_dram.ap())
            nc.sync.dma_start(out=xrows[4:5, :], in_=ones_dram.ap())
            eps_sb = cpool.tile([128, 1], f32, tag="eps")
            nc.gpsimd.memset(eps_sb[:], EPS)

            # PE warmup
            psw = pp1.tile([P, 2, TW], f32, tag="s1", name="warm")
            for _ in range(4):
                nc.tensor.matmul(
                    out=psw[:, 0, 0:128], lhsT=w1_sb[:],
                    rhs=w1_sb[:, 0:128], start=True, stop=True,
                )

            # big weights
            w2a_sb = cpool.tile([128, 8, 2, MROWS], fp8e4, tag="w2a")
            nc.sync.dma_start(
                out=w2a_sb[:],
                in_=w2a_dram.ap().rearrange("v k two m -> k v two m"),
            )
            w2b_sb = cpool.tile([128, 8, 2, MROWS], fp8e5, tag="w2b")
            nc.sync.dma_start(
                out=w2b_sb[:],
                in_=w2b_dram.ap().rearrange("v k two m -> k v two m"),
            )

            # ---- pointwise loads + prep (overlaps loop) ----
            x_pw = pw.tile([P, F], f32, tag="x")
            xview = x_dram.ap().rearrange(
                "(sp g pr hh c) -> sp pr g (hh c)", sp=4, g=8, pr=4, hh=2
            )
            for s in range(NS):
                nc.sync.dma_start(out=x_pw[s * 32:(s + 1) * 32, :], in_=xview[s])
            xb_pw = pw.tile([P, F], bf16, tag="xb")
            for s in range(NS):
                nc.gpsimd.dma_start(
                    out=xb_pw[s * 32:(s + 1) * 32, :], in_=xview[s]
                )
            xe_pw = pw.tile([P, F], f32, tag="xe")
            nc.vector.tensor_scalar(
                xe_pw[:], xb_pw[:], 63.75, EPS, Alu.min, Alu.add
            )
            Qb = tmp.tile([P, F], f32, tag="ta", name="Qb")
            nc.scalar.activation(Qb[:], xe_pw[:], AF.Copy,
                                 bias=MAGIC - 0.5, scale=0.5)
            q5 = pw.tile([P, F], f32, tag="q5")
            nc.scalar.activation(q5[:], Qb[:], AF.Copy, bias=-MAGIC + 0.5)
            vc_pw = pw.tile([P, F], f16, tag="vc")
            nc.vector.scalar_tensor_tensor(
                vc_pw[:], q5[:], -2.0, x_pw[:], Alu.mult, Alu.add
            )
            # relu-cube ingredients (ready before tail)
            w_pw = pw.tile([P, F], f16, tag="w")
            nc.vector.tensor_scalar(w_pw[:], vc_pw[:], 0.0, 1.0, Alu.max, Alu.mult)
            w2_pw = pw.tile([P, F], f16, tag="w2")
            nc.vector.tensor_tensor(out=w2_pw[:], in0=w_pw[:], in1=w_pw[:], op=Alu.mult)
            v2_pw = pw.tile([P, F], f16, tag="v2")
            nc.vector.tensor_tensor(out=v2_pw[:], in0=vc_pw[:], in1=vc_pw[:], op=Alu.mult)

            staging = stg.tile([MROWS, NG, 2 * TW], f16, tag="stg")

            # ---- pipelined pair loop: 32 pairs of 2 taus ----
            # pair idx pi = G*4 + h*2 + pp covers taus (G, pr=2pp, h), (G, pr=2pp+1, h)
            pairs = [
                (G, h, pp)
                for G in range(NG)
                for h in range(2)
                for pp in range(2)
            ]
            NPAIR = len(pairs)
            ps1_p = [None] * NPAIR
            sig_p = [None] * NPAIR
            ps2_g = [None] * NG

            def taus_of(pi):
                G, h, pp = pairs[pi]
                return [(G, 2 * pp + d, h) for d in range(2)]

            def s0(pi):  # 2 MM1s into one double tile
                ps1 = pp1.tile([P, 2, TW], f32, tag="s1", name=f"ps1_{pi}")
                ps1_p[pi] = ps1
                for d, (G, pr, h) in enumerate(taus_of(pi)):
                    tau = G * 8 + pr * 2 + h
                    nc.tensor.matmul(
                        out=ps1[:, d], lhsT=w1_sb[:],
                        rhs=xrows[:, tau * TW:(tau + 1) * TW],
                        start=True, stop=True,
                    )

            def s1(pi):  # one 1024-wide indicator
                idx = pairs[pi][0] * 4 + pairs[pi][1] * 2 + pairs[pi][2]
                sig = sigp.tile([P, 2, TW], fp8e4, tag="sg", name=f"sig{pi}")
                sig_p[pi] = sig
                src = ps1_p[pi][:].rearrange("p d c -> p (d c)")
                dst = sig[:].rearrange("p d c -> p (d c)")
                if ENG_ACT[idx]:
                    nc.scalar.activation(dst, src, AF.Sign, bias=eps_sb[:])
                else:
                    nc.vector.tensor_scalar(
                        dst, src, -EPS, 2.0, Alu.is_ge, Alu.mult
                    )

            def s2(pi):  # 4 MM2s + evac at G end
                G, h, pp = pairs[pi]
                idx = G * 4 + h * 2 + pp
                ci = 0 if ENG_ACT[idx] else 1
                if ps2_g[G] is None:
                    ps2_g[G] = pp2.tile([MROWS, 2, TW], f32, tag="s2", name=f"ps2_{G}")
                ps2 = ps2_g[G]
                for d, (G_, pr, h_) in enumerate(taus_of(pi)):
                    wi = ci * 4 + pr
                    sg = sig_p[pi][:, d]
                    rhs2a = sg.unsqueeze(1).broadcast_to([P, 2, TW])
                    nc.tensor.matmul(
                        out=ps2[:, h], lhsT=w2a_sb[:, wi], rhs=rhs2a,
                        start=(pr == 0), stop=False, perf_mode=DR,
                    )
                    rhs2b = sg.bitcast(fp8e5).unsqueeze(1).broadcast_to([P, 2, TW])
                    nc.tensor.matmul(
                        out=ps2[:, h], lhsT=w2b_sb[:, wi], rhs=rhs2b,
                        start=False, stop=(pr == 3), perf_mode=DR,
                    )
                if pp == 1:
                    dst = staging[:, G, h * TW:(h + 1) * TW]
                    src = ps2[:, h]
                    if (G + h) % 2 == 0:
                        nc.scalar.copy(out=dst, in_=src)
                    else:
                        nc.vector.tensor_copy(out=dst, in_=src)

            SKEW = 2
            for t in range(NPAIR + SKEW):
                if 0 <= t - 1 < NPAIR:
                    s1(t - 1)
                if t < NPAIR:
                    s0(t)
                if 0 <= t - SKEW < NPAIR:
                    s2(t - SKEW)

            # ---- reloads + horner ----
            g_pw = pw.tile([P, NV, F], f16, tag="gpw")
            gk = [g_pw[:, v, :] for v in range(NV)]
            for i, val in enumerate((2, 0, 3, 1, 4)):
                eng = nc.gpsimd if i % 2 == 1 else nc.sync
                eng.dma_start(
                    out=g_pw[:, val, :],
                    in_=staging[val * 16:(val + 1) * 16],
                )
            t1 = tmp.tile([P, F], f16, tag="ta", name="t1")
            nc.vector.tensor_tensor(out=t1[:], in0=v2_pw[:], in1=gk[2], op=Alu.mult)
            e0 = tmp.tile([P, F], f16, tag="tb", name="e0")
            nc.vector.tensor_tensor(out=e0[:], in0=gk[0], in1=t1[:], op=Alu.add)
            t2 = tmp.tile([P, F], f16, tag="tc", name="t2")
            nc.vector.tensor_tensor(out=t2[:], in0=v2_pw[:], in1=gk[3], op=Alu.mult)
            e1 = tmp.tile([P, F], f16, tag="td", name="e1")
            nc.vector.tensor_tensor(out=e1[:], in0=gk[1], in1=t2[:], op=Alu.add)
            t4 = tmp.tile([P, F], f16, tag="ta", name="t4")
            nc.vector.tensor_tensor(out=t4[:], in0=gk[4], in1=w_pw[:], op=Alu.mult)
            t3 = tmp.tile([P, F], f16, tag="tc", name="t3")
            nc.vector.tensor_tensor(out=t3[:], in0=vc_pw[:], in1=e1[:], op=Alu.mult)
            y0 = tmp.tile([P, F], f16, tag="td", name="y0")
            nc.vector.tensor_tensor(out=y0[:], in0=e0[:], in1=t3[:], op=Alu.add)
            u = tmp.tile([P, F], f16, tag="tb", name="u")
            nc.vector.tensor_tensor(out=u[:], in0=t4[:], in1=w2_pw[:], op=Alu.mult)
            y16 = pw.tile([P, F], f16, tag="y")
            nc.vector.tensor_tensor(out=y16[:], in0=y0[:], in1=u[:], op=Alu.add)
            nc.sync.dma_start(out=y_dram.ap(), in_=y16[:])

    nc.compile()
    return nc


def get_program():
    if "prog" not in _PROG_CACHE:
        _PROG_CACHE["prog"] = _build_program()
    return _PROG_CACHE["prog"]


def make_in_maps(x: np.ndarray, coefs: np.ndarray):
    import ml_dtypes

    bf = ml_dtypes.bfloat16
    w1, w2a, w2b = _host_arrays(coefs)
    ones1 = np.ones((1, SLOTN), bf)
    shards = np.asarray(x, np.float32).reshape(N_CORES, N)
    return [
        {"x": shards[i].copy(), "w1": w1, "w2a": w2a, "w2b": w2b, "ones1": ones1}
        for i in range(N_CORES)
    ]


def kernel(x, coefs, knot_vector=None, _trace: bool = False):
    from concourse.bass_utils import run_bass_kernel_spmd

    nc = get_program()
    in_maps = make_in_maps(x, coefs)
    res = run_bass_kernel_spmd(nc, in_maps, list(range(N_CORES)), trace=_trace)
    out = np.concatenate(
        [_unpermute_y(r["out"].astype(np.float32)) for r in res.results]
    )
    if _trace:
        return out, res
    return out
